# revision 1
# baseline (speedup 1.0000x reference)
"""Channel-attention MultiHeadAttention kernel for Trainium2 (8 NeuronCores).

Math: for this module, attention is over channels (d x d per head) with the
spatial dim N = H*W as the contraction axis. The whole module collapses:
  G = x @ x.T (256x256 Gram), s = rowsum(x)
  S = scale * [Wq|bq] @ [[G, s],[s^T, N]] @ [Wk|bk]^T   (only 8 diag 32x32 blocks)
  attn = softmax(S_blocks)
  Wfinal = WoutP @ blockdiag(attn) @ Wv ;  bfinal = bout + WoutP @ blockdiag(attn) @ bv
  out = Wfinal @ x + bfinal
Sharding: data-parallel over batch B=8, one batch element per core. No collectives.

I/O in fp16 both ways (host casts): x arrives fp16 (8 MB/core) and is DMA'd
straight into persistent SBUF tiles; out leaves fp16 (8 MB/core) and the host
upcasts. This halves HBM traffic vs fp32 and removes all on-device casts of x.
"""

import numpy as np
from contextlib import ExitStack

B, C, H, W = 8, 256, 128, 128
N = H * W          # 16384
NH, D = 8, 32      # heads, head dim
SCALE = D ** -0.5
CH = 512           # phase C column chunk (one PSUM bank fp32)
SUB = 128          # transpose subchunk
NCORES = 8

TRACE = False      # test.py may set kernel.TRACE = True
LAST_RESULTS = {}  # exec_time_ns etc. for test.py

_CACHE = {}


def _build_real(repeat=1, stop_after=None, lch=2048, nxt=8, pst_bufs=4,
                act_slots=(1, 4, 6), psc_bufs=6, sch=2048, skip_gram=False,
                skip_transpose=False, gram_lag=4, out_sync_only=False,
                psb_bufs=4, psm_bufs=2, pool_cast=0, warm=(0, 64), unroll=1, in_split=True, out3=False):
    # stop_after: None | 'A' | 'tiny' - truncated builds for simulator bisection
    import concourse.bacc as bacc
    import concourse.mybir as mybir
    import concourse.tile as tile

    dt = mybir.dt
    f32, f16 = dt.float32, dt.float16
    Exp = mybir.ActivationFunctionType.Exp
    Ident = mybir.ActivationFunctionType.Identity
    X = mybir.AxisListType.X

    nc = bacc.Bacc(trn_type="TRN2")

    x_d = nc.dram_tensor("xb", [C, N], f16, kind="ExternalInput")
    qaT_d = nc.dram_tensor("qaT", [257, 256], f16, kind="ExternalInput")
    kaT_d = nc.dram_tensor("kaT", [257, 256], f16, kind="ExternalInput")
    wva_d = nc.dram_tensor("wva", [256, 257], f16, kind="ExternalInput")
    wpT_d = nc.dram_tensor("wpT", [256, 256], f16, kind="ExternalInput")
    bout_d = nc.dram_tensor("boutc", [256, 1], f32, kind="ExternalInput")
    id_d = nc.dram_tensor("ident", [128, 128], f16, kind="ExternalInput")
    gd_d = nc.dram_tensor("gdiag", [256, 257], f32, kind="ExternalInput")
    gd1_d = nc.dram_tensor("gdiag1", [128, 129], f32, kind="ExternalInput")
    corr_d = nc.dram_tensor("corr", [256, 32], f32, kind="ExternalInput")
    out_d = nc.dram_tensor("out", [C, N], f16, kind="ExternalOutput")

    with ExitStack() as top:
        tc = top.enter_context(tile.TileContext(nc))
        persist = top.enter_context(tc.tile_pool(name="persist", bufs=1))

        # x in SBUF split into two n-halves per channel half, so the
        # loop-carried WAR resolves per half: next-iter loads of the first
        # half may overlap phase C reads of the second half
        NHF = N // 2
        x16 = [[persist.tile([128, NHF], f16, tag=f"x16_{i}_{h}", name=f"x16_{i}_{h}")
                for h in range(2)] for i in range(2)]

        def xsl(i, gsl):
            h = gsl.start // NHF
            assert gsl.stop <= (h + 1) * NHF
            return x16[i][h][:, gsl.start - h * NHF:gsl.stop - h * NHF]

        qaT_t = [persist.tile([128, 256], f16, tag="qaT0", name="qaT0"),
                 persist.tile([128, 256], f16, tag="qaT1", name="qaT1"),
                 persist.tile([1, 256], f16, tag="qaT2", name="qaT2")]
        kaT_t = [persist.tile([128, 256], f16, tag="kaT0", name="kaT0"),
                 persist.tile([128, 256], f16, tag="kaT1", name="kaT1"),
                 persist.tile([1, 256], f16, tag="kaT2", name="kaT2")]
        wva_t = [persist.tile([128, 257], f16, tag="wva0", name="wva0"),
                 persist.tile([128, 257], f16, tag="wva1", name="wva1")]
        wpT_t = [persist.tile([128, 256], f16, tag="wpT0", name="wpT0"),
                 persist.tile([128, 256], f16, tag="wpT1", name="wpT1")]
        bout_t = [persist.tile([128, 1], f32, tag="bout0", name="bout0"),
                  persist.tile([128, 1], f32, tag="bout1", name="bout1")]
        id16 = persist.tile([128, 128], f16, tag="id16", name="id16")
        gd_t = [persist.tile([128, 257], f32, tag="gd0", name="gd0"),
                persist.tile([128, 129], f32, tag="gd1", name="gd1")]
        corr_t = [persist.tile([128, 32], f32, tag="corr0", name="corr0"),
                  persist.tile([128, 32], f32, tag="corr1", name="corr1")]

        nc.sync.dma_start(out=id16, in_=id_d.ap())
        nc.gpsimd.dma_start(out=qaT_t[0], in_=qaT_d.ap()[0:128, :])
        nc.gpsimd.dma_start(out=qaT_t[1], in_=qaT_d.ap()[128:256, :])
        nc.gpsimd.dma_start(out=qaT_t[2], in_=qaT_d.ap()[256:257, :])
        nc.gpsimd.dma_start(out=kaT_t[0], in_=kaT_d.ap()[0:128, :])
        nc.gpsimd.dma_start(out=kaT_t[1], in_=kaT_d.ap()[128:256, :])
        nc.gpsimd.dma_start(out=kaT_t[2], in_=kaT_d.ap()[256:257, :])
        nc.gpsimd.dma_start(out=wva_t[0], in_=wva_d.ap()[0:128, :])
        nc.gpsimd.dma_start(out=wva_t[1], in_=wva_d.ap()[128:256, :])
        nc.gpsimd.dma_start(out=wpT_t[0], in_=wpT_d.ap()[0:128, :])
        nc.gpsimd.dma_start(out=wpT_t[1], in_=wpT_d.ap()[128:256, :])
        nc.gpsimd.dma_start(out=bout_t[0], in_=bout_d.ap()[0:128, :])
        nc.gpsimd.dma_start(out=bout_t[1], in_=bout_d.ap()[128:256, :])
        nc.gpsimd.dma_start(out=gd_t[0], in_=gd_d.ap()[0:128, :])
        nc.gpsimd.dma_start(out=gd_t[1], in_=gd1_d.ap())
        nc.gpsimd.dma_start(out=corr_t[0], in_=corr_d.ap()[0:128, :])
        nc.gpsimd.dma_start(out=corr_t[1], in_=corr_d.ap()[128:256, :])

        # outputs of the tiny stage used by phase C
        wf16 = [persist.tile([128, 256], f16, tag=f"wf16_{k}", name=f"wf16_{k}") for k in range(2)]
        bf_col = [persist.tile([128, 1], f32, tag=f"bf{m}", name=f"bf{m}") for m in range(2)]

        # transposed-x tiles (allocated up-front so the ones columns can be
        # written once, outside the repeat loop)
        xts_all = [persist.tile([128, 4, 257], f16, tag=f"xt{j}", name=f"xt{j}")
                   for j in range(N // (4 * SUB))]
        for j in range(N // (4 * SUB)):
            nc.vector.memset(xts_all[j][:, :, 256:257], 1.0)

        # ---------------- pools hoisted out of the repeat loop ----------------
        psA = top.enter_context(tc.tile_pool(name="psA", bufs=1, space="PSUM"))
        psB = top.enter_context(tc.tile_pool(name="psB", bufs=psb_bufs, space="PSUM"))
        psM = top.enter_context(tc.tile_pool(name="psM", bufs=psm_bufs, space="PSUM"))
        tp = top.enter_context(tc.tile_pool(name="tinysb", bufs=1))
        ost = top.enter_context(tc.tile_pool(name="ost", bufs=4))

        G_ps = [psA.tile([128, 257], f32, tag="g0", name="g0"),
                psA.tile([128, 129], f32, tag="g1", name="g1")]

        # `repeat` counts LOGICAL iterations; with unroll>1 the For_i body
        # holds `unroll` copies so the scheduler can software-pipeline across
        # the pair (the back-edge itself never overlaps)
        assert repeat % unroll == 0 or repeat == 1
        rep = ExitStack()
        if repeat > 1:
            rep.enter_context(tc.For_i(0, repeat // unroll, 1))
        for _u in range(unroll if repeat > 1 else 1):

            NG = N // (4 * SUB)     # 32 groups of 4 chunks
            xts = xts_all
            LCH = lch
            GRP = 4 * SUB
            GPL = LCH // GRP        # groups per LCH block

            def gram_group(g):
                for q in range(4):
                    n0 = (g * 4 + q) * SUB
                    first, last = (n0 == 0), (n0 == N - SUB)
                    nc.tensor.matmul(G_ps[0][:], lhsT=xts[g][:, q, 0:128],
                                     rhs=xts[g][:, q, 0:257],
                                     start=first, stop=last)
                    nc.tensor.matmul(G_ps[1][:], lhsT=xts[g][:, q, 128:256],
                                     rhs=xts[g][:, q, 128:257],
                                     start=first, stop=last)

            def pe_warm(n):
                # dependency-free PE work (transpose the identity into a
                # scratch PSUM slot) to hold the HAM p-state at full clock
                # through windows where PE would otherwise idle
                for _ in range(n // 8):
                    wps = psB.tile([128, 4, 256], f16, tag="tps", name="warm")
                    for q in range(4):
                        nc.tensor.transpose(wps[:, q, 0:128], id16[:], id16[:])
                        nc.tensor.transpose(wps[:, q, 128:256], id16[:], id16[:])

            t = 0
            for co in range(N // LCH):
                sl = slice(co * LCH, (co + 1) * LCH)
                # fp16 x straight into the persistent SBUF copy (no cast);
                # two HWDGE queues so aggregate DMA can reach the HBM cap
                nc.sync.dma_start(out=xsl(0, sl), in_=x_d.ap()[0:128, sl])
                (nc.scalar if in_split else nc.sync).dma_start(
                    out=xsl(1, sl), in_=x_d.ap()[128:256, sl])
                if skip_transpose:
                    continue
                for gi in range(GPL):
                    g0 = co * LCH + gi * GRP
                    xt = xts[t]
                    tp_ps = psB.tile([128, 4, 256], f16, tag="tps", name="tps")
                    for q in range(4):
                        n0 = g0 + q * SUB
                        nc.tensor.transpose(tp_ps[:, q, 0:128],
                                            xsl(0, slice(n0, n0 + SUB)), id16[:])
                        nc.tensor.transpose(tp_ps[:, q, 128:256],
                                            xsl(1, slice(n0, n0 + SUB)), id16[:])
                    # one strided PSUM -> SBUF copy per 4 chunks
                    if t % 8 in act_slots:
                        nc.scalar.copy(out=xt[:, :, 0:256], in_=tp_ps[:, :, :])
                    else:
                        nc.vector.tensor_copy(out=xt[:, :, 0:256],
                                              in_=tp_ps[:, :, :])
                    t += 1
                    # software-pipelined gram, gram_lag groups behind
                    if not skip_gram and gram_lag > 0 and t > gram_lag:
                        gram_group(t - gram_lag - 1)
            if not skip_gram:
                for g in range((max(0, t - gram_lag) if gram_lag > 0 else 0), NG):
                    gram_group(g)
            if stop_after is None and warm[0]:
                pe_warm(warm[0])


            if stop_after != "A":
                # ------- Tiny stage (fp16 matmuls @1cy/row; G diag pre-subtracted,
                # exact correction  scale*N*Qa@Ka^T  added back from host) -------
                pst = psM

                # Ga tiles: [G - N*I | s] rows 0:256 plus the [s^T | 0] row, fp16
                Ga = [tp.tile([128, 257], f16, tag=f"Ga{k}", name=f"Ga{k}") for k in range(2)]
                nc.vector.tensor_sub(Ga[0][:], G_ps[0][:], gd_t[0][:])
                # G10 = G01^T (symmetry); G11/s1 from the 129-col accumulator
                g10_ps = pst.tile([128, 128], f16, tag="tinyps", name="g10ps")
                nc.tensor.transpose(g10_ps[:], Ga[0][:, 128:256], id16[:])
                nc.scalar.copy(out=Ga[1][:, 0:128], in_=g10_ps[:])
                nc.vector.tensor_sub(Ga[1][:, 128:257], G_ps[1][:, 0:129], gd_t[1][:])
                Ga2 = tp.tile([1, 257], f16, tag="Ga2", name="Ga2")
                for k in range(2):
                    srow_ps = pst.tile([1, 128], f16, tag="tinyps", name="tinyps")
                    nc.tensor.transpose(srow_ps[:], Ga[k][:, 256:257], id16[:])
                    nc.vector.tensor_copy(out=Ga2[0:1, 128 * k:128 * (k + 1)],
                                          in_=srow_ps[:])
                nc.vector.memset(Ga2[0:1, 256:257], 0.0)
                GaK = [Ga[0], Ga[1], Ga2]

                # T2 = Ga' @ KaT  (257 x 256), M-tiles over rows of T2
                t2s = [tp.tile([128, 256], f16, tag="t2s0", name="t2s0"),
                       tp.tile([128, 256], f16, tag="t2s1", name="t2s1"),
                       tp.tile([1, 256], f16, tag="t2s2", name="t2s2")]
                for m in range(3):
                    msl = slice(256, 257) if m == 2 else slice(128 * m, 128 * (m + 1))
                    t2_ps = pst.tile([1 if m == 2 else 128, 256], f32, tag="tinyps", name="tinyps")
                    for k in range(3):
                        nc.tensor.matmul(t2_ps[:], lhsT=GaK[k][:, msl], rhs=kaT_t[k][:],
                                         start=(k == 0), stop=(k == 2))
                    if m == 1:
                        nc.scalar.copy(out=t2s[m][:], in_=t2_ps[:])
                    else:
                        nc.vector.tensor_copy(out=t2s[m][:], in_=t2_ps[:])

                # S_full' = QaT.T @ T2 (256 x 256) in PSUM (full-M matmuls;
                # small-M tile_position modes are slow on real HW)
                SF = []
                for m in range(2):
                    sf_ps = pst.tile([128, 256], f32, tag="tinyps", name=f"sfps{m}")
                    msl = slice(128 * m, 128 * (m + 1))
                    for k in range(3):
                        nc.tensor.matmul(sf_ps[:], lhsT=qaT_t[k][:, msl], rhs=t2s[k][:],
                                         start=(k == 0), stop=(k == 2))
                    SF.append(sf_ps)
                # add exact correction; then softmax pieces
                Sst = [tp.tile([128, 32], f32, tag=f"sst{q}", name=f"sst{q}") for q in range(2)]
                for h in range(NH):
                    q, po = h // 4, (h % 4) * 32
                    nc.vector.tensor_add(Sst[q][po:po + 32, :],
                                         SF[q][po:po + 32, h * 32:(h + 1) * 32],
                                         corr_t[q][po:po + 32, :])

                # exp(S - max) written straight into block-diagonal fp16 tiles
                abd = [tp.tile([128, 128], f16, tag=f"abd{q}", name=f"abd{q}") for q in range(2)]
                for q in range(2):
                    nc.gpsimd.memset(abd[q][:], 0.0)
                nm = [tp.tile([128, 1], f32, tag=f"nm{q}", name=f"nm{q}") for q in range(2)]
                for q in range(2):
                    nc.vector.reduce_max(out=nm[q][:], in_=Sst[q][:], axis=X, negate=True)
                for h in range(NH):
                    q, po = h // 4, (h % 4) * 32
                    nc.scalar.activation(out=abd[q][po:po + 32, po:po + 32],
                                         in_=Sst[q][po:po + 32, :], func=Exp,
                                         bias=nm[q][po:po + 32, :], scale=1.0)
                # row-sums of exp via the block-diag tiles (zeros add nothing)
                rc = [tp.tile([128, 1], f32, tag=f"rc{q}", name=f"rc{q}") for q in range(2)]
                for q in range(2):
                    sm = tp.tile([128, 1], f32, tag=f"sm{q}", name=f"sm{q}")
                    nc.vector.reduce_sum(out=sm[:], in_=abd[q][:], axis=X)
                    nc.vector.reciprocal(out=rc[q][:], in_=sm[:])
                abdT = [tp.tile([128, 128], f16, tag=f"abdT{q}", name=f"abdT{q}") for q in range(2)]
                for q in range(2):
                    tq_ps = pst.tile([128, 128], f16, tag="tinyps", name="tinyps")
                    nc.tensor.transpose(tq_ps[:], abd[q][:], id16[:])
                    if q == 1:
                        nc.scalar.copy(out=abdT[q][:], in_=tq_ps[:])
                    else:
                        nc.vector.tensor_copy(out=abdT[q][:], in_=tq_ps[:])

                # Weff_aug = blockdiag(exp) @ [Wv | bv], rows scaled by 1/sum
                weff = [tp.tile([128, 257], f16, tag=f"weff{k}", name=f"weff{k}") for k in range(2)]
                for k in range(2):
                    we_ps = pst.tile([128, 257], f32, tag="tinyps", name="tinyps")
                    nc.tensor.matmul(we_ps[:], lhsT=abdT[k][:], rhs=wva_t[k][:],
                                     start=True, stop=True)
                    nc.vector.tensor_scalar_mul(weff[k][:], we_ps[:], rc[k][:])

                # Wfinal^T = Weff[:, :256] as lhsT against WoutP^T; cast to fp16
                for m in range(2):
                    msl = slice(128 * m, 128 * (m + 1))
                    wf_ps = pst.tile([128, 256], f32, tag="tinyps", name=f"wfps{m}")
                    for k in range(2):
                        nc.tensor.matmul(wf_ps[:], lhsT=weff[k][:, msl], rhs=wpT_t[k][:],
                                         start=(k == 0), stop=(k == 1))
                    if m == 1:
                        nc.scalar.copy(out=wf16[m][:], in_=wf_ps[:])
                    else:
                        nc.vector.tensor_copy(out=wf16[m][:], in_=wf_ps[:])

                # bfinal = bout + WoutP @ beff   (beff = Weff[:, 256])
                for m in range(2):
                    msl = slice(128 * m, 128 * (m + 1))
                    bf_ps = pst.tile([128, 1], f32, tag="tinyps", name="tinyps")
                    for k in range(2):
                        nc.tensor.matmul(bf_ps[:], lhsT=wpT_t[k][:, msl],
                                         rhs=weff[k][:, 256:257],
                                         start=(k == 0), stop=(k == 1))
                    nc.vector.tensor_add(bf_col[m][:], bf_ps[:], bout_t[m][:])


        # ---------------- Phase C: out = Wfinal @ x + bfinal ----------------
        if True:
          if stop_after not in ("A", "tiny"):
              SCH = sch
              for co in range(N // SCH):
                  slo = slice(co * SCH, (co + 1) * SCH)
                  for m in range(2):
                      o_sb = ost.tile([128, SCH], f16, tag=f"osb{m}", name=f"osb{m}")
                      for hp in range(SCH // CH // 2):
                          # pair of column chunks, k-interleaved so each weight
                          # slice stays loaded for 2 matmuls (halves LDWEIGHTS)
                          sls = [slice(co * SCH + (2 * hp + j) * CH,
                                       co * SCH + (2 * hp + j + 1) * CH)
                                 for j in range(2)]
                          # widen the effective PSUM ring to 6 slots: tiny's
                          # psM banks are idle during phase C, so every third
                          # accumulator borrows one (bank round-trip latency,
                          # not PE streaming, paces this loop)
                          ops = []
                          for j in range(2):
                              oc = co * 8 + m * 4 + 2 * hp + j
                              if oc % 3 == 2:
                                  ops.append(psM.tile([128, CH], f32,
                                                      tag="tinyps", name="ops"))
                              else:
                                  ops.append(psB.tile([128, CH], f32,
                                                      tag="tps", name="ops"))
                          for k in range(2):
                              for j in range(2):
                                  nc.tensor.matmul(
                                      ops[j][:],
                                      lhsT=wf16[k][:, 128 * m:128 * (m + 1)],
                                      rhs=xsl(k, sls[j]),
                                      start=(k == 0), stop=(k == 1))
                          for j in range(2):
                              h = 2 * hp + j
                              dst = o_sb[:, h * CH:(h + 1) * CH]
                              ci = co * 8 + m * 4 + h
                              if pool_cast and ci % pool_cast == pool_cast - 1:
                                  nc.gpsimd.tensor_scalar_add(dst, ops[j][:], bf_col[m][:])
                              elif ci % 2 == 0:
                                  nc.vector.tensor_scalar_add(dst, ops[j][:], bf_col[m][:])
                              else:
                                  nc.scalar.activation(
                                      out=dst, in_=ops[j][:], func=Ident,
                                      bias=bf_col[m][:], scale=1.0)
                      oi = co * 2 + m
                      if out3 and oi in (0, 8):
                          eng = nc.sync
                      elif out3 and oi in (4, 12):
                          eng = nc.scalar
                      elif out3:
                          eng = nc.gpsimd
                      else:
                          eng = nc.sync if m == 0 else nc.gpsimd
                      eng.dma_start(out=out_d.ap()[128 * m:128 * (m + 1), slo],
                                    in_=o_sb[:])
              if warm[1]:
                  pe_warm(warm[1])
        rep.close()

    nc.finalize()
    return nc


def _host_prep(Wqkv, bqkv, Wout, bout):
    Wq, Wk, Wv = Wqkv[:C], Wqkv[C:2 * C], Wqkv[2 * C:]
    bq, bk, bv = bqkv[:C], bqkv[C:2 * C], bqkv[2 * C:]
    qa = np.concatenate([Wq, bq[:, None]], axis=1) * SCALE      # (256, 257)
    ka = np.concatenate([Wk, bk[:, None]], axis=1)              # (256, 257)
    qaT = np.ascontiguousarray(qa.T)                            # (257, 256)
    kaT = np.ascontiguousarray(ka.T)
    wva = np.concatenate([Wv, bv[:, None]], axis=1)             # (256, 257)
    r = np.arange(C)
    WoutP = Wout[:, (r % D) * NH + (r // D)]                    # (256, 256)
    wpT = np.ascontiguousarray(WoutP.T)
    gdiag = np.zeros((256, 257), dtype=np.float32)
    gdiag[np.arange(256), np.arange(256)] = float(N)
    gdiag1 = np.zeros((128, 129), dtype=np.float32)
    gdiag1[np.arange(128), np.arange(128)] = float(N)
    corr_full = float(N) * (qa @ ka.T)                          # (256, 256) fp32
    corr = np.zeros((256, 32), dtype=np.float32)
    for h in range(NH):
        corr[h * D:(h + 1) * D, :] = corr_full[h * D:(h + 1) * D,
                                               h * D:(h + 1) * D]
    return {
        "qaT": qaT.astype(np.float16), "kaT": kaT.astype(np.float16),
        "wva": np.ascontiguousarray(wva, dtype=np.float16),
        "wpT": wpT.astype(np.float16),
        "boutc": np.ascontiguousarray(bout[:, None], dtype=np.float32),
        "ident": np.eye(128, dtype=np.float16),
        "gdiag": gdiag, "gdiag1": gdiag1, "corr": corr,
    }


def kernel(x, Wqkv, bqkv, Wout, bout, num_heads):
    from concourse.bass_utils import run_bass_kernel_spmd

    assert int(num_heads) == NH
    x16 = np.asarray(x, dtype=np.float16)
    shared = _host_prep(
        np.asarray(Wqkv, dtype=np.float32), np.asarray(bqkv, dtype=np.float32),
        np.asarray(Wout, dtype=np.float32), np.asarray(bout, dtype=np.float32))

    if "nc" not in _CACHE:
        _CACHE["nc"] = _build_real()
    nc = _CACHE["nc"]

    in_maps = [{"xb": np.ascontiguousarray(x16[c].reshape(C, N)), **shared}
               for c in range(NCORES)]

    res = run_bass_kernel_spmd(nc, in_maps, core_ids=list(range(NCORES)),
                               trace=TRACE)
    LAST_RESULTS["exec_time_ns"] = res.exec_time_ns
    out = np.stack([res.results[c]["out"].astype(np.float32)
                    for c in range(NCORES)])
    return out.reshape(B, C, H, W)

